# revision 57
# baseline (speedup 1.0000x reference)
"""Causal multi-head attention (B=4, T=2048, D=1024, H=16, d_h=64) on 8 trn2 cores.

Sharding: data-parallel over batch (4) x tensor-parallel over head halves (2).
Core c handles batch c//2, heads [8*(c%2), 8*(c%2)+8), i.e. output columns
[512*(c%2), 512*(c%2)+512) of out[c//2].

Per-core kernel, all matmul operands bf16 (fp32 PSUM accumulate; fp32r at
N>=256 is already 1 cycle/row, but bf16 removes the narrow-matmul penalty
and halves input DMA):
  - x arrives HOST-TRANSPOSED as xT [1024, 2048] bf16, so no PE transposes.
  - A few PE warmup matmuls on the (tiny, early) consts tile keep the
    tensor-engine clock ramping while real operands are in flight on DMA.
  - V proj: v_nat [128t, 512] = xT_chunk^T @ Wv per t-tile; stored bf16 with
    an interleaved ones column per head ([128, 8*65]) so the AV matmul also
    produces the softmax denominator.
  - Q/K proj: qT/kT [128, 2048] bf16 per head-pair group g
    (lhsT=W chunk, rhs=xT).
  - Attention per (q-block j of 512, group g), k-tile i (block-causal):
      scores sT[128k, q] x 2 heads -> one 2-bank PSUM tile; for diagonal
      tiles only the unmasked column range [128m:512] is computed, and the
      causal mask is a PE accumulate of ident^T @ (-1e30 triangle) - off
      the DVE so nothing else sits on the exp -> AV chain.
      p = exp(s/8) via ONE strided ScalarE activation -> bf16 (no max
      subtraction: |s/8| is small). The exp stream is the attention-phase
      co-bottleneck (ACT is 1 elem/lane/cycle regardless of dtype).
      AV in NATURAL layout: ctx[128q, 65] += pt_chunk^T @ [v_h|1] per
      128-query chunk, skipping fully-masked chunk x k-tile combos; all
      four chunks of a head share ONE PSUM accumulation group (start marks
      the whole 2KB zero-region pending-zero so each chunk's first write
      auto-zeroes; stop rides the last write into the bank). AV runs TWO
      k-tiles late so PE never waits out the exp latency.
    Epilogue: fast PSUM->SBUF copy (frees the ctx bank for the next block),
    then reciprocal of the l column + per-partition scale + DMA out - pure
    DVE/DMA, chunked into the next block's k-loop behind its last AV.

All pools share ONE scope (PSUM: proj ring 2x1 + scores ring 2x2 + ctx
2x1 = 8 banks exactly), so no cross-phase barrier is needed. Overlap
strategy: only V + QK(group 0) run up-front; QK groups 1-3 are emitted as
CONTIGUOUS 9-instruction chunks at attention block boundaries (g-major
block order), hiding ~46us of projection under the ScalarE exp stream,
which is the attention-phase pacer. Contiguity is critical: fine-grained
(per-iteration) interleave of projection matmuls was tried and inflates
EVERY matmul ~50ns (breaks PE weight-load/stream overlap). Same effect at
micro scale: on diagonal tiles both heads' score matmuls are emitted
back-to-back, THEN both mask accumulates, to avoid thrashing the kT/ident
weight loads.
"""

import os
import sys

for _p in ("/opt/trn_rl_repo", "/root/.axon_site/_ro/trn_rl_repo"):
    if os.path.isdir(_p) and _p not in sys.path:
        sys.path.insert(0, _p)

import ml_dtypes
import numpy as np

import concourse.mybir as mybir  # noqa: E402
import concourse.tile as tile  # noqa: E402
from concourse import bacc  # noqa: E402
from concourse.bass_utils import run_bass_kernel_spmd  # noqa: E402

F32 = mybir.dt.float32
BF16 = mybir.dt.bfloat16
BF_NP = ml_dtypes.bfloat16

P = 128
T = 2048
DIN = 1024
DL = 512          # local d_out per core
HL = 8            # local heads
DH = 64
NT = T // P       # 16 t-tiles
NDI = DIN // P    # 8 d_in tiles
NG = DL // P      # 4 head-pair groups
NJ = T // 512     # 4 q blocks
SCALE = 1.0 / np.sqrt(DH)

Exp = mybir.ActivationFunctionType.Exp


def _build():
    nc = bacc.Bacc(None, target_bir_lowering=False)
    xt = nc.dram_tensor("xt", [DIN, T], BF16, kind="ExternalInput")
    wq = nc.dram_tensor("wq", [DIN, DL], BF16, kind="ExternalInput")
    wk = nc.dram_tensor("wk", [DIN, DL], BF16, kind="ExternalInput")
    wv = nc.dram_tensor("wv", [DIN, DL], BF16, kind="ExternalInput")
    # cols 0-127: identity; cols 128-255: -1e30 where query f < key p
    consts_d = nc.dram_tensor("consts", [P, 2 * P], BF16, kind="ExternalInput")
    out = nc.dram_tensor("out", [T, DL], F32, kind="ExternalOutput")

    xt_r = xt[:].rearrange("(k p) t -> k p t", p=P)
    w_r = {n: w[:].rearrange("(k p) n -> k p n", p=P) for n, w in
           (("q", wq), ("k", wk), ("v", wv))}
    # out rows 512j + 128c + p
    out_r = out[:].rearrange("(j c p) n -> j p c n", j=NJ, c=4)

    with tile.TileContext(nc) as tc:
        with (
            tc.tile_pool(name="const", bufs=1) as const,
            tc.tile_pool(name="qk", bufs=4) as qk_pool,
            tc.tile_pool(name="v", bufs=1) as v_pool,
            tc.tile_pool(name="x", bufs=1) as x_pool,
            tc.tile_pool(name="w", bufs=1) as w_pool,
        ):
            consts = const.tile([P, 2 * P], BF16)
            nc.sync.dma_start(out=consts, in_=consts_d[:])
            ident, negtri = consts[:, 0:P], consts[:, P:2 * P]
            ones_f = const.tile([P, HL], F32)
            nc.vector.memset(ones_f, 1.0)
            v_sb = [v_pool.tile([P, HL * (DH + 1)], BF16, tag=f"v{t_}",
                                name=f"v{t_}") for t_ in range(NT)]
            xt_sb = [x_pool.tile([P, T], BF16, tag=f"x{di}", name=f"xt{di}")
                     for di in range(NDI)]
            w_sb = {which: [w_pool.tile([P, DL], BF16, tag=f"w{which}{di}",
                                        name=f"w{which}{di}")
                            for di in range(NDI)]
                    for which in ("v", "q", "k")}

            # DMA priority order: wv + first xt column-chunk gate the first
            # V-proj matmuls; later xt chunks and wq/wk follow.
            for di in range(NDI):
                nc.sync.dma_start(out=w_sb["v"][di], in_=w_r["v"][di])
            for di in range(NDI):
                nc.sync.dma_start(out=xt_sb[di][:, 0:128], in_=xt_r[di][:, 0:128])
            for di in range(NDI):
                nc.sync.dma_start(out=xt_sb[di][:, 128:512],
                                  in_=xt_r[di][:, 128:512])
            for cb in range(1, 4):
                for di in range(NDI):
                    nc.sync.dma_start(out=xt_sb[di][:, 512 * cb:512 * cb + 512],
                                      in_=xt_r[di][:, 512 * cb:512 * cb + 512])
            for which in ("q", "k"):
                for di in range(NDI):
                    nc.sync.dma_start(out=w_sb[which][di], in_=w_r[which][di])

            qTs, kTs = {}, {}
            with (
                tc.tile_pool(name="ps_b", bufs=2, space="PSUM") as ps_b,
                tc.tile_pool(name="pt", bufs=4) as pt_pool,
                tc.tile_pool(name="cp", bufs=2) as cp_pool,
                tc.tile_pool(name="ob", bufs=3) as ob_pool,
                tc.tile_pool(name="rec", bufs=3) as rec_pool,
                tc.tile_pool(name="ps_s", bufs=2, space="PSUM") as ps_s,
                tc.tile_pool(name="ps_ctx", bufs=1, space="PSUM") as ps_ctx,
            ):
                # PE warmup: dummy matmuls on the (tiny, early) consts tile
                # keep the tensor-engine clock ramping while the first real
                # operands are still in flight on DMA.
                warm = ps_b.tile([P, DL], F32, tag="b", name="warm")
                for _ in range(26):
                    nc.tensor.matmul(warm[:, 0:2 * P], ident, consts,
                                     start=True, stop=True)
                # ---- V projection: natural layout + interleaved ones ----
                for ti in range(NT):
                    ps = ps_b.tile([P, DL], F32, tag="b", name="psv")
                    for di in range(NDI):
                        nc.tensor.matmul(
                            ps, xt_sb[di][:, P * ti:P * ti + P], w_sb["v"][di],
                            start=(di == 0), stop=(di == NDI - 1))
                    vt = v_sb[ti]
                    if ti % 2 == 0:
                        nc.vector.tensor_copy(
                            vt[:].rearrange("p (h e) -> p h e",
                                            e=DH + 1)[:, :, DH],
                            ones_f)
                        nc.vector.tensor_copy(
                            vt[:].rearrange("p (h e) -> p h e",
                                            e=DH + 1)[:, :, 0:DH],
                            ps[:].rearrange("p (h d) -> p h d", d=DH))
                    else:
                        nc.scalar.copy(
                            vt[:].rearrange("p (h e) -> p h e",
                                            e=DH + 1)[:, :, DH],
                            ones_f)
                        nc.scalar.copy(
                            vt[:].rearrange("p (h e) -> p h e",
                                            e=DH + 1)[:, :, 0:DH],
                            ps[:].rearrange("p (h d) -> p h d", d=DH))

                # ---- Q/K projections: qT/kT [128, 2048] per group g ----
                # one (which, g, tb) chunk = 8 accumulate matmuls + a copy;
                # groups 0-2 run up-front, group 3's chunks are emitted as
                # CONTIGUOUS groups at attention block boundaries, hiding
                # under the ScalarE exp stream (phase C has ~20us of PE
                # slack). Contiguity matters: fine-grained interleave breaks
                # PE weight-load/stream overlap.
                def proj_chunk(which, g, tb):
                    dst = qTs[g] if which == "q" else kTs[g]
                    ps = ps_b.tile([P, DL], F32, tag="b", name="pspj")
                    for di in range(NDI):
                        nc.tensor.matmul(
                            ps, w_sb[which][di][:, P * g:P * g + P],
                            xt_sb[di][:, 512 * tb:512 * tb + 512],
                            start=(di == 0), stop=(di == NDI - 1))
                    if tb % 2 == 0:
                        nc.vector.tensor_copy(
                            dst[:, 512 * tb:512 * tb + 512], ps)
                    else:
                        nc.scalar.copy(
                            dst[:, 512 * tb:512 * tb + 512], ps)

                for which, dsts in (("q", qTs), ("k", kTs)):
                    for g in range(NG):
                        dsts[g] = qk_pool.tile([P, T], BF16, tag=f"{which}T",
                                               name=f"{which}T{g}")
                for which in ("q", "k"):
                    for tb in range(4):
                        proj_chunk(which, 0, tb)
                proj_pending = [(which, g, tb) for g in range(1, NG)
                                for which in ("q", "k") for tb in range(4)]
                def make_epi(j, g, ctx, split_dma=False):
                    # chunk 1 (per sg): fast PSUM->SBUF copy frees the ctx
                    # bank for the next block's AV; chunk 2: normalize from
                    # SBUF (recip + muls on DVE) + DMA out.
                    state = {}

                    def cpy(sg, ctx=ctx):
                        cp = cp_pool.tile([P, 4 * (DH + 1)], F32, tag=f"cp{sg}",
                                          name="cp")
                        nc.vector.tensor_copy(
                            cp[:].rearrange("p (c e) -> p c e", e=DH + 1),
                            ctx[sg][:].rearrange(
                                "p (c e) -> p c e", e=P)[:, :, 0:DH + 1])
                        state[sg] = cp

                    def norm(sg, j=j, g=g, split_dma=split_dma):
                        cp = state[sg]
                        hl = 2 * g + sg
                        rec = rec_pool.tile([P, 4], F32, tag="rec", name="rec")
                        nc.vector.reciprocal(
                            rec, cp[:].rearrange(
                                "p (c e) -> p c e", e=DH + 1)[:, :, DH])
                        ob = ob_pool.tile([P, 4 * DH], F32, tag="ob", name="ob")
                        for c in range(4):
                            nc.vector.tensor_scalar_mul(
                                ob[:, DH * c:DH * c + DH],
                                cp[:, (DH + 1) * c:(DH + 1) * c + DH],
                                rec[:, c:c + 1])
                        if split_dma:
                            # nothing left to overlap the final writeback
                            # with - spread it over 4 queues
                            for c in range(4):
                                nc.sync.dma_start(
                                    out=out_r[j][:, c, DH * hl:DH * hl + DH],
                                    in_=ob[:, DH * c:DH * c + DH])
                        else:
                            nc.sync.dma_start(
                                out=out_r[j][:, :, DH * hl:DH * hl + DH],
                                in_=ob[:].rearrange("p (c d) -> p c d", d=DH))
                    return [lambda: cpy(0), lambda: cpy(1),
                            lambda: norm(0), lambda: norm(1)]

                av_queue = []
                epi_pending = []
                # g-major so each group's boundary-inserted projection
                # drains before its blocks; within g, the short block
                # follows the long one so its epilogue drains during a
                # dense k-loop
                order = [(j, g) for g in range(NG) for j in (3, 0, 2, 1)]
                for bi, (j, g) in enumerate(order):
                    # one contiguous QK projection chunk per boundary (skip
                    # the first: no scalar backlog yet) - a single ~2.2us
                    # chunk hides under the ~2us exp backlog, two would
                    # leave the scalar idle; more chunks go at mid-points of
                    # long blocks below. The g-guard is a force-drain safety
                    # for anything the current block's group still needs.
                    npop = 1 if bi >= 1 else 0
                    while proj_pending and (npop > 0
                                            or proj_pending[0][1] <= g):
                        proj_chunk(*proj_pending.pop(0))
                        npop -= 1
                    nk = 4 * (j + 1)
                    mids = ({nk // 3, (2 * nk) // 3} if j == 3 else
                            {nk // 2} if j >= 1 else set())
                    # ctx chunk c lives at cols [128c, 128c+65) (bank-aligned
                    # tile); col 128c+64 is the denominator. Allocated at
                    # i==0 AFTER the previous block's epilogue copies are
                    # emitted, so the bufs=1 ring reuse sees those readers.
                    ctx = [None, None]
                    for i in range(nk):
                        m = i - 4 * j          # >= 0 on diagonal tiles
                        mm = max(m, 0)
                        st = ps_s.tile([P, 2 * DL], F32, tag="s", name="st")
                        for sg in range(2):
                            nc.tensor.matmul(
                                st[:, DL * sg + P * mm:DL * sg + DL],
                                kTs[g][DH * sg:DH * sg + DH, P * i:P * i + P],
                                qTs[g][DH * sg:DH * sg + DH,
                                       DL * j + P * mm:DL * j + DL],
                                start=True, stop=(m < 0))
                        if m >= 0:
                            # causal mask for the diagonal 128x128 chunk as a
                            # PE accumulate (keeps DVE off the exp -> AV
                            # chain); both heads' masks back-to-back so the
                            # ident weight load isn't thrashed between kT
                            # loads
                            for sg in range(2):
                                nc.tensor.matmul(
                                    st[:, DL * sg + P * m:DL * sg + P * m + P],
                                    ident, negtri, start=False, stop=True)
                        pt = pt_pool.tile([P, 2 * DL], BF16, tag="pt", name="pt")
                        if m >= 0:
                            # one activation for both heads via a strided AP
                            nc.scalar.activation(
                                pt[:].rearrange("p (s f) -> p s f",
                                                s=2)[:, :, P * m:DL],
                                st[:].rearrange("p (s f) -> p s f",
                                                s=2)[:, :, P * m:DL],
                                Exp, scale=float(SCALE))
                        else:
                            nc.scalar.activation(pt, st, Exp, scale=float(SCALE))
                        # AV runs TWO k-tiles late so PE never waits out the
                        # exp latency even in the steady state.
                        if len(av_queue) >= 2:
                            av_queue.pop(0)()
                        # i==0: the previous block's last AV is still queued,
                        # so its epilogue (which reads that ctx) must wait;
                        # from i==1 the queue head was that AV, drained above.
                        if i >= 1:
                            for _ in range(2):
                                if epi_pending:
                                    epi_pending.pop(0)()
                        if i == 1 or (i == 0 and nk == 1):
                            for sg in range(2):
                                ctx[sg] = ps_ctx.tile(
                                    [P, 4 * P], F32, tag=f"c{sg}",
                                    name=f"ctx{sg}")
                        # mid-block projection chunk: hides under the local
                        # exp backlog just like the boundary ones
                        if i in mids and proj_pending:
                            proj_chunk(*proj_pending.pop(0))

                        def av(i=i, pt=pt, mm=mm, ctx=ctx, nk=nk, g=g):
                            # One accumulation group per ctx PSUM bank: start
                            # marks the whole 2KB zero-region pending-zero, so
                            # each chunk's first write auto-zeroes; stop goes
                            # on the last write into the bank.
                            for sg in range(2):
                                hl = 2 * g + sg
                                vsl = v_sb[i][:, (DH + 1) * hl:
                                              (DH + 1) * (hl + 1)]
                                for c in range(mm, 4):
                                    nc.tensor.matmul(
                                        ctx[sg][:, P * c:P * c + DH + 1],
                                        pt[:, DL * sg + P * c:
                                           DL * sg + P * c + P],
                                        vsl,
                                        start=(i == 0 and c == 0),
                                        stop=(i == nk - 1 and c == 3))
                        av_queue.append(av)
                    epi_pending += make_epi(j, g, ctx)
                for a in av_queue:
                    a()
                for e in epi_pending:
                    e()
    nc.compile()
    return nc


_NC = None


def _get_nc():
    global _NC
    if _NC is None:
        _NC = _build()
    return _NC


# cols 0-127: identity; cols 128-255: -1e30 where query f < key p (causal
# additive mask for the diagonal 128x128 chunk)
_CONSTS = np.concatenate([
    np.eye(P, dtype=np.float32),
    np.where(np.arange(P)[None, :] < np.arange(P)[:, None],
             np.float32(-1e30), np.float32(0.0)),
], axis=1).astype(BF_NP)


def run(inputs, **spmd_kwargs):
    x, W_q, W_k, W_v = (inputs["x"], inputs["W_q"], inputs["W_k"], inputs["W_v"])
    nc = _get_nc()
    in_maps = []
    for c in range(8):
        b, half = divmod(c, 2)
        sl = slice(DL * half, DL * half + DL)
        in_maps.append({
            "xt": np.ascontiguousarray(
                np.asarray(x[b], dtype=np.float32).T).astype(BF_NP),
            "wq": np.ascontiguousarray(np.asarray(W_q[:, sl], dtype=np.float32)
                                       ).astype(BF_NP),
            "wk": np.ascontiguousarray(np.asarray(W_k[:, sl], dtype=np.float32)
                                       ).astype(BF_NP),
            "wv": np.ascontiguousarray(np.asarray(W_v[:, sl], dtype=np.float32)
                                       ).astype(BF_NP),
            "consts": _CONSTS,
        })
    res = run_bass_kernel_spmd(nc, in_maps, core_ids=list(range(8)), **spmd_kwargs)
    B = x.shape[0]
    full = np.empty((B, T, 2 * DL), dtype=np.float32)
    for c in range(8):
        b, half = divmod(c, 2)
        full[b][:, DL * half:DL * half + DL] = res.results[c]["out"]
    return full, res


def kernel(**inputs):
    return run(inputs)[0]


if __name__ == "__main__":
    rng = np.random.default_rng(0)
    ins = {
        "x": rng.standard_normal((4, T, DIN), dtype=np.float32),
        "W_q": (rng.random((DIN, 2 * DL), dtype=np.float32) - 0.5) / 16,
        "W_k": (rng.random((DIN, 2 * DL), dtype=np.float32) - 0.5) / 16,
        "W_v": (rng.random((DIN, 2 * DL), dtype=np.float32) - 0.5) / 16,
    }
    o = kernel(**ins)
    print("ran ok", o.shape, o.dtype)


# revision 58
# speedup vs baseline: 1.0082x; 1.0082x over previous
"""Causal multi-head attention (B=4, T=2048, D=1024, H=16, d_h=64) on 8 trn2 cores.

Sharding: data-parallel over batch (4) x tensor-parallel over head halves (2).
Core c handles batch c//2, heads [8*(c%2), 8*(c%2)+8), i.e. output columns
[512*(c%2), 512*(c%2)+512) of out[c//2].

Per-core kernel, all matmul operands bf16 (fp32 PSUM accumulate; fp32r at
N>=256 is already 1 cycle/row, but bf16 removes the narrow-matmul penalty
and halves input DMA):
  - x arrives HOST-TRANSPOSED as xT [1024, 2048] bf16, so no PE transposes.
  - A few PE warmup matmuls on the (tiny, early) consts tile keep the
    tensor-engine clock ramping while real operands are in flight on DMA.
  - V proj: v_nat [128t, 512] = xT_chunk^T @ Wv per t-tile; stored bf16 with
    an interleaved ones column per head ([128, 8*65]) so the AV matmul also
    produces the softmax denominator.
  - Q/K proj: qT/kT [128, 2048] bf16 per head-pair group g
    (lhsT=W chunk, rhs=xT).
  - Attention per (q-block j of 512, group g), k-tile i (block-causal):
      scores sT[128k, q] x 2 heads -> one 2-bank PSUM tile; for diagonal
      tiles only the unmasked column range [128m:512] is computed, and the
      causal mask is a PE accumulate of ident^T @ (-1e30 triangle) - off
      the DVE so nothing else sits on the exp -> AV chain.
      p = exp(s/8) via ONE strided ScalarE activation -> bf16 (no max
      subtraction: |s/8| is small). The exp stream is the attention-phase
      co-bottleneck (ACT is 1 elem/lane/cycle regardless of dtype).
      AV in NATURAL layout: ctx[128q, 65] += pt_chunk^T @ [v_h|1] per
      128-query chunk, skipping fully-masked chunk x k-tile combos; all
      four chunks of a head share ONE PSUM accumulation group (start marks
      the whole 2KB zero-region pending-zero so each chunk's first write
      auto-zeroes; stop rides the last write into the bank). AV runs TWO
      k-tiles late so PE never waits out the exp latency.
    Epilogue: fast PSUM->SBUF copy (frees the ctx bank for the next block),
    then reciprocal of the l column + per-partition scale + DMA out - pure
    DVE/DMA, chunked into the next block's k-loop behind its last AV.

All pools share ONE scope (PSUM: proj ring 2x1 + scores ring 2x2 + ctx
2x1 = 8 banks exactly), so no cross-phase barrier is needed. Overlap
strategy: only V + QK(group 0) run up-front; QK groups 1-3 are emitted as
CONTIGUOUS 9-instruction chunks at attention block boundaries (g-major
block order), hiding ~46us of projection under the ScalarE exp stream,
which is the attention-phase pacer. Contiguity is critical: fine-grained
(per-iteration) interleave of projection matmuls was tried and inflates
EVERY matmul ~50ns (breaks PE weight-load/stream overlap). Same effect at
micro scale: on diagonal tiles both heads' score matmuls are emitted
back-to-back, THEN both mask accumulates, to avoid thrashing the kT/ident
weight loads.
"""

import os
import sys

for _p in ("/opt/trn_rl_repo", "/root/.axon_site/_ro/trn_rl_repo"):
    if os.path.isdir(_p) and _p not in sys.path:
        sys.path.insert(0, _p)

import ml_dtypes
import numpy as np

import concourse.mybir as mybir  # noqa: E402
import concourse.tile as tile  # noqa: E402
from concourse import bacc  # noqa: E402
from concourse.bass_utils import run_bass_kernel_spmd  # noqa: E402

F32 = mybir.dt.float32
BF16 = mybir.dt.bfloat16
BF_NP = ml_dtypes.bfloat16

P = 128
T = 2048
DIN = 1024
DL = 512          # local d_out per core
HL = 8            # local heads
DH = 64
NT = T // P       # 16 t-tiles
NDI = DIN // P    # 8 d_in tiles
NG = DL // P      # 4 head-pair groups
NJ = T // 512     # 4 q blocks
SCALE = 1.0 / np.sqrt(DH)

Exp = mybir.ActivationFunctionType.Exp


def _build():
    nc = bacc.Bacc(None, target_bir_lowering=False)
    xt = nc.dram_tensor("xt", [DIN, T], BF16, kind="ExternalInput")
    wq = nc.dram_tensor("wq", [DIN, DL], BF16, kind="ExternalInput")
    wk = nc.dram_tensor("wk", [DIN, DL], BF16, kind="ExternalInput")
    wv = nc.dram_tensor("wv", [DIN, DL], BF16, kind="ExternalInput")
    # cols 0-127: identity; cols 128-255: -1e30 where query f < key p
    consts_d = nc.dram_tensor("consts", [P, 2 * P], BF16, kind="ExternalInput")
    out = nc.dram_tensor("out", [T, DL], F32, kind="ExternalOutput")

    xt_r = xt[:].rearrange("(k p) t -> k p t", p=P)
    w_r = {n: w[:].rearrange("(k p) n -> k p n", p=P) for n, w in
           (("q", wq), ("k", wk), ("v", wv))}
    # out rows 512j + 128c + p
    out_r = out[:].rearrange("(j c p) n -> j p c n", j=NJ, c=4)

    with tile.TileContext(nc) as tc:
        with (
            tc.tile_pool(name="const", bufs=1) as const,
            tc.tile_pool(name="qk", bufs=4) as qk_pool,
            tc.tile_pool(name="v", bufs=1) as v_pool,
            tc.tile_pool(name="x", bufs=1) as x_pool,
            tc.tile_pool(name="w", bufs=1) as w_pool,
        ):
            consts = const.tile([P, 2 * P], BF16)
            nc.sync.dma_start(out=consts, in_=consts_d[:])
            ident, negtri = consts[:, 0:P], consts[:, P:2 * P]
            ones_f = const.tile([P, HL], F32)
            nc.vector.memset(ones_f, 1.0)
            v_sb = [v_pool.tile([P, HL * (DH + 1)], BF16, tag=f"v{t_}",
                                name=f"v{t_}") for t_ in range(NT)]
            xt_sb = [x_pool.tile([P, T], BF16, tag=f"x{di}", name=f"xt{di}")
                     for di in range(NDI)]
            w_sb = {which: [w_pool.tile([P, DL], BF16, tag=f"w{which}{di}",
                                        name=f"w{which}{di}")
                            for di in range(NDI)]
                    for which in ("v", "q", "k")}

            # DMA priority order: wv + first xt column-chunk gate the first
            # V-proj matmuls; later xt chunks and wq/wk follow.
            for di in range(NDI):
                nc.sync.dma_start(out=w_sb["v"][di], in_=w_r["v"][di])
            for di in range(NDI):
                nc.sync.dma_start(out=xt_sb[di][:, 0:128], in_=xt_r[di][:, 0:128])
            for di in range(NDI):
                nc.sync.dma_start(out=xt_sb[di][:, 128:512],
                                  in_=xt_r[di][:, 128:512])
            for cb in range(1, 4):
                for di in range(NDI):
                    nc.sync.dma_start(out=xt_sb[di][:, 512 * cb:512 * cb + 512],
                                      in_=xt_r[di][:, 512 * cb:512 * cb + 512])
            for which in ("q", "k"):
                for di in range(NDI):
                    nc.sync.dma_start(out=w_sb[which][di], in_=w_r[which][di])

            qTs, kTs = {}, {}
            with (
                tc.tile_pool(name="ps_b", bufs=2, space="PSUM") as ps_b,
                tc.tile_pool(name="pt", bufs=4) as pt_pool,
                tc.tile_pool(name="cp", bufs=2) as cp_pool,
                tc.tile_pool(name="ob", bufs=3) as ob_pool,
                tc.tile_pool(name="rec", bufs=3) as rec_pool,
                tc.tile_pool(name="ps_s", bufs=2, space="PSUM") as ps_s,
                tc.tile_pool(name="ps_ctx", bufs=1, space="PSUM") as ps_ctx,
            ):
                # PE warmup: dummy matmuls on the (tiny, early) consts tile
                # keep the tensor-engine clock ramping while the first real
                # operands are still in flight on DMA.
                warm = ps_b.tile([P, DL], F32, tag="b", name="warm")
                for _ in range(26):
                    nc.tensor.matmul(warm[:, 0:2 * P], ident, consts,
                                     start=True, stop=True)
                # ---- V projection: natural layout + interleaved ones ----
                for ti in range(NT):
                    ps = ps_b.tile([P, DL], F32, tag="b", name="psv")
                    for di in range(NDI):
                        nc.tensor.matmul(
                            ps, xt_sb[di][:, P * ti:P * ti + P], w_sb["v"][di],
                            start=(di == 0), stop=(di == NDI - 1))
                    vt = v_sb[ti]
                    if ti % 2 == 0:
                        nc.vector.tensor_copy(
                            vt[:].rearrange("p (h e) -> p h e",
                                            e=DH + 1)[:, :, DH],
                            ones_f)
                        nc.vector.tensor_copy(
                            vt[:].rearrange("p (h e) -> p h e",
                                            e=DH + 1)[:, :, 0:DH],
                            ps[:].rearrange("p (h d) -> p h d", d=DH))
                    else:
                        nc.scalar.copy(
                            vt[:].rearrange("p (h e) -> p h e",
                                            e=DH + 1)[:, :, DH],
                            ones_f)
                        nc.scalar.copy(
                            vt[:].rearrange("p (h e) -> p h e",
                                            e=DH + 1)[:, :, 0:DH],
                            ps[:].rearrange("p (h d) -> p h d", d=DH))

                # ---- Q/K projections: qT/kT [128, 2048] per group g ----
                # one (which, g, tb) chunk = 8 accumulate matmuls + a copy;
                # groups 0-2 run up-front, group 3's chunks are emitted as
                # CONTIGUOUS groups at attention block boundaries, hiding
                # under the ScalarE exp stream (phase C has ~20us of PE
                # slack). Contiguity matters: fine-grained interleave breaks
                # PE weight-load/stream overlap.
                def proj_chunk(which, g, tb):
                    dst = qTs[g] if which == "q" else kTs[g]
                    ps = ps_b.tile([P, DL], F32, tag="b", name="pspj")
                    for di in range(NDI):
                        nc.tensor.matmul(
                            ps, w_sb[which][di][:, P * g:P * g + P],
                            xt_sb[di][:, 512 * tb:512 * tb + 512],
                            start=(di == 0), stop=(di == NDI - 1))
                    if tb % 2 == 0:
                        nc.vector.tensor_copy(
                            dst[:, 512 * tb:512 * tb + 512], ps)
                    else:
                        nc.scalar.copy(
                            dst[:, 512 * tb:512 * tb + 512], ps)

                for which, dsts in (("q", qTs), ("k", kTs)):
                    for g in range(NG):
                        dsts[g] = qk_pool.tile([P, T], BF16, tag=f"{which}T",
                                               name=f"{which}T{g}")
                for which in ("q", "k"):
                    for tb in range(4):
                        proj_chunk(which, 0, tb)
                proj_pending = [(which, g, tb) for g in range(1, NG)
                                for which in ("q", "k") for tb in range(4)]
                def make_epi(j, g, ctx, split_dma=False):
                    # chunk 1 (per sg): fast PSUM->SBUF copy frees the ctx
                    # bank for the next block's AV; chunk 2: normalize from
                    # SBUF (recip + muls on DVE) + DMA out.
                    state = {}

                    def cpy(sg, ctx=ctx):
                        cp = cp_pool.tile([P, 4 * (DH + 1)], F32, tag=f"cp{sg}",
                                          name="cp")
                        nc.vector.tensor_copy(
                            cp[:].rearrange("p (c e) -> p c e", e=DH + 1),
                            ctx[sg][:].rearrange(
                                "p (c e) -> p c e", e=P)[:, :, 0:DH + 1])
                        state[sg] = cp

                    def norm(sg, j=j, g=g, split_dma=split_dma):
                        cp = state[sg]
                        hl = 2 * g + sg
                        rec = rec_pool.tile([P, 4], F32, tag="rec", name="rec")
                        nc.vector.reciprocal(
                            rec, cp[:].rearrange(
                                "p (c e) -> p c e", e=DH + 1)[:, :, DH])
                        ob = ob_pool.tile([P, 4 * DH], F32, tag="ob", name="ob")
                        for c in range(4):
                            nc.vector.tensor_scalar_mul(
                                ob[:, DH * c:DH * c + DH],
                                cp[:, (DH + 1) * c:(DH + 1) * c + DH],
                                rec[:, c:c + 1])
                        if split_dma:
                            # nothing left to overlap the final writeback
                            # with - spread it over 4 queues
                            for c in range(4):
                                nc.sync.dma_start(
                                    out=out_r[j][:, c, DH * hl:DH * hl + DH],
                                    in_=ob[:, DH * c:DH * c + DH])
                        else:
                            nc.sync.dma_start(
                                out=out_r[j][:, :, DH * hl:DH * hl + DH],
                                in_=ob[:].rearrange("p (c d) -> p c d", d=DH))
                    return [lambda: cpy(0), lambda: cpy(1),
                            lambda: norm(0), lambda: norm(1)]

                av_queue = []
                epi_pending = []
                # g-major so each group's boundary-inserted projection
                # drains before its blocks; within g, the short block
                # follows the long one so its epilogue drains during a
                # dense k-loop
                order = [(j, g) for g in range(NG) for j in (3, 0, 2, 1)]
                for bi, (j, g) in enumerate(order):
                    # two contiguous QK projection chunks per boundary
                    # (skip the first boundary: no scalar backlog yet);
                    # the tail of the while is a force-drain safety for
                    # anything the current block's group still needs
                    npop = 2 if bi >= 1 else 0
                    while proj_pending and (npop > 0
                                            or proj_pending[0][1] <= g):
                        proj_chunk(*proj_pending.pop(0))
                        npop -= 1
                    nk = 4 * (j + 1)
                    # ctx chunk c lives at cols [128c, 128c+65) (bank-aligned
                    # tile); col 128c+64 is the denominator. Allocated at
                    # i==0 AFTER the previous block's epilogue copies are
                    # emitted, so the bufs=1 ring reuse sees those readers.
                    ctx = [None, None]
                    for i in range(nk):
                        m = i - 4 * j          # >= 0 on diagonal tiles
                        mm = max(m, 0)
                        st = ps_s.tile([P, 2 * DL], F32, tag="s", name="st")
                        for sg in range(2):
                            nc.tensor.matmul(
                                st[:, DL * sg + P * mm:DL * sg + DL],
                                kTs[g][DH * sg:DH * sg + DH, P * i:P * i + P],
                                qTs[g][DH * sg:DH * sg + DH,
                                       DL * j + P * mm:DL * j + DL],
                                start=True, stop=(m < 0))
                        if m >= 0:
                            # causal mask for the diagonal 128x128 chunk as a
                            # PE accumulate (keeps DVE off the exp -> AV
                            # chain); both heads' masks back-to-back so the
                            # ident weight load isn't thrashed between kT
                            # loads
                            for sg in range(2):
                                nc.tensor.matmul(
                                    st[:, DL * sg + P * m:DL * sg + P * m + P],
                                    ident, negtri, start=False, stop=True)
                        pt = pt_pool.tile([P, 2 * DL], BF16, tag="pt", name="pt")
                        if m >= 0:
                            # one activation for both heads via a strided AP
                            nc.scalar.activation(
                                pt[:].rearrange("p (s f) -> p s f",
                                                s=2)[:, :, P * m:DL],
                                st[:].rearrange("p (s f) -> p s f",
                                                s=2)[:, :, P * m:DL],
                                Exp, scale=float(SCALE))
                        else:
                            nc.scalar.activation(pt, st, Exp, scale=float(SCALE))
                        # AV runs TWO k-tiles late so PE never waits out the
                        # exp latency even in the steady state.
                        if len(av_queue) >= 2:
                            av_queue.pop(0)()
                        # i==0: the previous block's last AV is still queued,
                        # so its epilogue (which reads that ctx) must wait;
                        # from i==1 the queue head was that AV, drained above.
                        if i >= 1:
                            for _ in range(2):
                                if epi_pending:
                                    epi_pending.pop(0)()
                        if i == 1 or (i == 0 and nk == 1):
                            for sg in range(2):
                                ctx[sg] = ps_ctx.tile(
                                    [P, 4 * P], F32, tag=f"c{sg}",
                                    name=f"ctx{sg}")

                        def av(i=i, pt=pt, mm=mm, ctx=ctx, nk=nk, g=g):
                            # One accumulation group per ctx PSUM bank: start
                            # marks the whole 2KB zero-region pending-zero, so
                            # each chunk's first write auto-zeroes; stop goes
                            # on the last write into the bank.
                            for sg in range(2):
                                hl = 2 * g + sg
                                vsl = v_sb[i][:, (DH + 1) * hl:
                                              (DH + 1) * (hl + 1)]
                                for c in range(mm, 4):
                                    nc.tensor.matmul(
                                        ctx[sg][:, P * c:P * c + DH + 1],
                                        pt[:, DL * sg + P * c:
                                           DL * sg + P * c + P],
                                        vsl,
                                        start=(i == 0 and c == 0),
                                        stop=(i == nk - 1 and c == 3))
                        av_queue.append(av)
                    epi_pending += make_epi(j, g, ctx)
                for a in av_queue:
                    a()
                for e in epi_pending:
                    e()
    nc.compile()
    return nc


_NC = None


def _get_nc():
    global _NC
    if _NC is None:
        _NC = _build()
    return _NC


# cols 0-127: identity; cols 128-255: -1e30 where query f < key p (causal
# additive mask for the diagonal 128x128 chunk)
_CONSTS = np.concatenate([
    np.eye(P, dtype=np.float32),
    np.where(np.arange(P)[None, :] < np.arange(P)[:, None],
             np.float32(-1e30), np.float32(0.0)),
], axis=1).astype(BF_NP)


def run(inputs, **spmd_kwargs):
    x, W_q, W_k, W_v = (inputs["x"], inputs["W_q"], inputs["W_k"], inputs["W_v"])
    nc = _get_nc()
    in_maps = []
    for c in range(8):
        b, half = divmod(c, 2)
        sl = slice(DL * half, DL * half + DL)
        in_maps.append({
            "xt": np.ascontiguousarray(
                np.asarray(x[b], dtype=np.float32).T).astype(BF_NP),
            "wq": np.ascontiguousarray(np.asarray(W_q[:, sl], dtype=np.float32)
                                       ).astype(BF_NP),
            "wk": np.ascontiguousarray(np.asarray(W_k[:, sl], dtype=np.float32)
                                       ).astype(BF_NP),
            "wv": np.ascontiguousarray(np.asarray(W_v[:, sl], dtype=np.float32)
                                       ).astype(BF_NP),
            "consts": _CONSTS,
        })
    res = run_bass_kernel_spmd(nc, in_maps, core_ids=list(range(8)), **spmd_kwargs)
    B = x.shape[0]
    full = np.empty((B, T, 2 * DL), dtype=np.float32)
    for c in range(8):
        b, half = divmod(c, 2)
        full[b][:, DL * half:DL * half + DL] = res.results[c]["out"]
    return full, res


def kernel(**inputs):
    return run(inputs)[0]


if __name__ == "__main__":
    rng = np.random.default_rng(0)
    ins = {
        "x": rng.standard_normal((4, T, DIN), dtype=np.float32),
        "W_q": (rng.random((DIN, 2 * DL), dtype=np.float32) - 0.5) / 16,
        "W_k": (rng.random((DIN, 2 * DL), dtype=np.float32) - 0.5) / 16,
        "W_v": (rng.random((DIN, 2 * DL), dtype=np.float32) - 0.5) / 16,
    }
    o = kernel(**ins)
    print("ran ok", o.shape, o.dtype)
